# revision 2
# baseline (speedup 1.0000x reference)
"""Causal attention with ALiBi for nn_CausalAttention (B=4, T=2048, C=1024,
16 heads) on 8 TRN2 NeuronCores.

Sharding: batch (4) x head-group (2 groups of 8 heads) -> 8 cores.
Host pre-casts x and weights to bf16 (and folds the 1/8 logit scale into
Wq), so the device does no casts and loads x exactly once.

Per core, three phases:
  1. Projections: one pass over x; per 512-t chunk compute q/k for all
     4 head-pairs and v, writing qT/kT in [d, t] layout (aug rows 64-71
     carry the one-hot / -slope*i ALiBi trick; the -slope*i bf16 error
     cancels exactly in softmax) and v in [t, h, d] layout.
  2. Attention, head-major with lag-1 PV: per (head, j-block) QK is
     computed only over the causal span i >= 128*jb, exp'd in <=1536-col
     groups (3 PSUM banks) with the exact +slope*j f32 bias, and the
     128x128 diagonal block masked by gpsimd affine_select. PV uses
     column-offset partial matmuls over the same spans; an appended ones
     column yields the softmax denominator. Normalization is batched per
     head: one [1,2048]->[128,16] DMA transpose + reciprocal + one
     partition_broadcast + one multiply.
  3. Output projection y_partial = oT.T @ Wo_rows, emitted in bf16;
     host sums the two head-group partials per batch in f32.
"""

import math

import numpy as np

import concourse.bass as bass
import concourse.mybir as mybir
import concourse.tile as tile
from concourse import bacc
from concourse.bass_utils import run_bass_kernel_spmd

B, T, C = 4, 2048, 1024
NH, HD = 16, 64
NHC = 8  # heads per core
NJB = T // 128  # 16 j-blocks
P = 128

f32 = mybir.dt.float32
bf16 = mybir.dt.bfloat16

LAST_RESULTS = None
_NC_CACHE = None


def get_slopes(n):
    def pow2(n):
        start = 2 ** (-(2 ** (-(math.log2(n) - 3))))
        return [start * start**i for i in range(n)]

    if math.log2(n).is_integer():
        return pow2(n)
    c = 2 ** math.floor(math.log2(n))
    return pow2(c) + get_slopes(2 * c)[0::2][: n - c]


# packed pT row offsets: row jb holds the causal span i in [128*jb, T),
# i.e. (16-jb)*128 columns, stored back-to-back.
OFFS = []
_o = 0
for _jb in range(NJB):
    OFFS.append(_o)
    _o += (NJB - _jb) * P
NPCOL = _o  # 17408


def build_kernel():
    nc = bacc.Bacc("TRN2", target_bir_lowering=False, debug=False, num_devices=8)

    xT_d = nc.dram_tensor("xb", [C, T], bf16, kind="ExternalInput").ap()
    wq_d = nc.dram_tensor("wq", [C, 512], bf16, kind="ExternalInput").ap()
    wk_d = nc.dram_tensor("wk", [C, 512], bf16, kind="ExternalInput").ap()
    wv_d = nc.dram_tensor("wv", [C, 512], bf16, kind="ExternalInput").ap()
    wo_d = nc.dram_tensor("wo", [512, C], bf16, kind="ExternalInput").ap()
    qaug_d = nc.dram_tensor("qaugb", [8, NHC, T], bf16, kind="ExternalInput").ap()
    kaug_d = nc.dram_tensor("kaugb", [8, NHC, T], bf16, kind="ExternalInput").ap()
    biasj_d = nc.dram_tensor("biasj", [P, NHC, NJB], f32, kind="ExternalInput").ap()
    y_d = nc.dram_tensor("y", [T, C], bf16, kind="ExternalOutput").ap()

    xT_r = xT_d.rearrange("(cb p) t -> p cb t", p=P)  # [128, 8, 2048]
    wq_r = wq_d.rearrange("(cb p) m -> p cb m", p=P)  # [128, 8, 512]
    wk_r = wk_d.rearrange("(cb p) m -> p cb m", p=P)
    wv_r = wv_d.rearrange("(cb p) m -> p cb m", p=P)
    wo_r = wo_d.rearrange("(mb p) n -> p mb n", p=P)  # [128, 4, 1024]
    y_r = y_d.rearrange("(tb p) c -> p tb c", p=P)  # [128, 16, 1024]

    with tile.TileContext(nc) as tc:
        with tc.tile_pool(name="persist", bufs=1) as persist:
            # ---- persistent tiles ----
            qT2 = persist.tile([72, NHC, T], bf16)
            kT2 = persist.tile([72, NHC, T], bf16)
            vaug = persist.tile([P, NJB, NHC, 66], bf16)
            # oT2[dh, head-pair m, chunk c, i] ; partitions 0-63 head 2m,
            # 64-127 head 2m+1
            oT2 = persist.tile([P, 4, 4, 512], bf16)
            biasj = persist.tile([P, NHC, NJB], f32)
            wo_t = persist.tile([P, 4, C], bf16)

            nc.gpsimd.memset(vaug[:, :, :, 64:66], 1.0)
            nc.sync.dma_start(biasj[:], biasj_d[:])
            nc.sync.dma_start(kT2[64:72, :, :], kaug_d[:])
            nc.sync.dma_start(qT2[64:72, :, :], qaug_d[:])
            nc.sync.dma_start(wo_t[:], wo_r[:])

            # ---- phase 1: projections (single pass over x) ----
            with (
                tc.tile_pool(name="wp", bufs=1) as wp,
                tc.tile_pool(name="xp", bufs=16) as xp,
                tc.tile_pool(name="psB", bufs=4, space="PSUM") as psB,
            ):
                wq_t = wp.tile([P, 8, 512], bf16)
                wk_t = wp.tile([P, 8, 512], bf16)
                wv_t = wp.tile([P, 8, 512], bf16)
                nc.sync.dma_start(wq_t[:], wq_r[:])
                nc.sync.dma_start(wk_t[:], wk_r[:])
                nc.sync.dma_start(wv_t[:], wv_r[:])

                for tck in range(4):
                    tsl = bass.ts(tck, 512)
                    xts = []
                    for c in range(8):
                        xt = xp.tile([P, 512], bf16, tag="xt")
                        nc.sync.dma_start(xt[:], xT_r[:, c, tsl])
                        xts.append(xt)
                    for m in range(4):
                        psq = psB.tile([P, 512], f32, tag="pb")
                        psk = psB.tile([P, 512], f32, tag="pb")
                        for c in range(8):
                            nc.tensor.matmul(
                                psq[:],
                                wq_t[:, c, bass.ts(m, P)],
                                xts[c][:],
                                start=(c == 0),
                                stop=(c == 7),
                            )
                            nc.tensor.matmul(
                                psk[:],
                                wk_t[:, c, bass.ts(m, P)],
                                xts[c][:],
                                start=(c == 0),
                                stop=(c == 7),
                            )
                        # even head: aligned ACT copy; odd head: DVE
                        # partition-shift copy (64-ch op may read any
                        # 64-part window, writes parts 0-63)
                        nc.scalar.activation(
                            qT2[0:64, 2 * m, tsl],
                            psq[0:64, :],
                            mybir.ActivationFunctionType.Copy,
                        )
                        nc.vector.tensor_copy(qT2[0:64, 2 * m + 1, tsl], psq[64:128, :])
                        nc.scalar.activation(
                            kT2[0:64, 2 * m, tsl],
                            psk[0:64, :],
                            mybir.ActivationFunctionType.Copy,
                        )
                        nc.vector.tensor_copy(kT2[0:64, 2 * m + 1, tsl], psk[64:128, :])
                    for tb in range(4):
                        psv = psB.tile([P, 512], f32, tag="pb")
                        for c in range(8):
                            nc.tensor.matmul(
                                psv[:],
                                xts[c][:, bass.ts(tb, P)],
                                wv_t[:, c, :],
                                start=(c == 0),
                                stop=(c == 7),
                            )
                        nc.vector.tensor_copy(
                            vaug[:, 4 * tck + tb, :, 0:64],
                            psv[:].rearrange("p (h d) -> p h d", h=NHC),
                        )

            # ---- phase 2: attention, head-major, lag-1 PV ----
            with (
                tc.tile_pool(name="psA", bufs=2, space="PSUM") as psA,
                tc.tile_pool(name="potp", bufs=2, space="PSUM") as potp,
                tc.tile_pool(name="pTp", bufs=2) as pTp,
                tc.tile_pool(name="pohp", bufs=2) as pohp,
                tc.tile_pool(name="bcp", bufs=1) as bcp,
                tc.tile_pool(name="srp", bufs=1) as srp,
                tc.tile_pool(name="rsp", bufs=2) as rsp,
            ):
                pT_of = {}

                def emit_qk(h):
                    pT = pTp.tile([P, NPCOL], bf16, tag="pT")
                    pT_of[h] = pT
                    for jb in range(NJB):
                        span = (NJB - jb) * P
                        off = 0
                        while off < span:
                            gw = min(1536, span - off)
                            ps = psA.tile([P, 1536], f32, tag="qk")
                            s = 0
                            while s < gw:
                                w = min(512, gw - s)
                                nc.tensor.matmul(
                                    ps[:, s : s + w],
                                    kT2[:, h, bass.ts(jb, P)],
                                    qT2[:, h, P * jb + off + s : P * jb + off + s + w],
                                    start=True,
                                    stop=True,
                                )
                                s += w
                            nc.scalar.activation(
                                pT[:, OFFS[jb] + off : OFFS[jb] + off + gw],
                                ps[:, 0:gw],
                                mybir.ActivationFunctionType.Exp,
                                bias=biasj[:, h, jb : jb + 1],
                                scale=1.0,
                            )
                            off += gw
                        # mask the 128x128 diagonal block: keep f - p >= 0
                        # (also kills Inf from masked overflow)
                        nc.gpsimd.affine_select(
                            pT[:, OFFS[jb] : OFFS[jb] + P],
                            pT[:, OFFS[jb] : OFFS[jb] + P],
                            pattern=[[1, P]],
                            compare_op=mybir.AluOpType.is_ge,
                            fill=0.0,
                            base=0,
                            channel_multiplier=-1,
                        )

                def emit_pv(h):
                    hp = (h % 2) * 64
                    hm = h // 2
                    pT = pT_of.pop(h)
                    poh = pohp.tile([65, 4, 512], f32, tag="poh")
                    for c in range(4):
                        pot = potp.tile([65, 512], f32, tag="pot")
                        njb = 4 * c + 4
                        for jb in range(njb):
                            w = min(512, (njb - jb) * P)
                            roff = 512 * c - P * jb if jb <= 4 * c else 0
                            nc.tensor.matmul(
                                pot[:, 512 - w : 512],
                                vaug[:, jb, h, 0:65],
                                pT[:, OFFS[jb] + roff : OFFS[jb] + roff + w],
                                start=(jb == 0),
                                stop=(jb == njb - 1),
                            )
                        nc.vector.tensor_copy(poh[:, c, :], pot[:])
                    # batched normalization for the whole head
                    rs = rsp.tile([P, 16], f32, tag="rs")
                    nc.sync.dma_start(rs[:], poh[64:65, :, :])
                    nc.vector.reciprocal(rs[:], rs[:])
                    sr = srp.tile([1, T], f32, tag="sr")
                    nc.sync.dma_start(sr[:], rs[:])
                    bc = bcp.tile([64, 4, 512], f32, tag="bc")
                    nc.gpsimd.partition_broadcast(bc[:], sr[0:1, :])
                    nc.vector.tensor_tensor(
                        oT2[hp : hp + 64, hm, :, :],
                        poh[0:64, :, :],
                        bc[:],
                        mybir.AluOpType.mult,
                    )

                for h in range(NHC):
                    emit_qk(h)
                    if h > 0:
                        emit_pv(h - 1)
                emit_pv(NHC - 1)

            # ---- phase 3: output projection ----
            with (
                tc.tile_pool(name="psY", bufs=2, space="PSUM") as psY,
                tc.tile_pool(name="ypool", bufs=4) as ypool,
            ):
                for tb in range(NJB):
                    for cc in range(2):
                        psy = psY.tile([P, 512], f32, tag="py")
                        for m in range(4):
                            nc.tensor.matmul(
                                psy[:],
                                oT2[:, m, tb // 4, bass.ts(tb % 4, P)],
                                wo_t[:, m, bass.ts(cc, 512)],
                                start=(m == 0),
                                stop=(m == 3),
                            )
                        ysb = ypool.tile([P, 512], bf16, tag="ysb")
                        if (tb + cc) % 2 == 0:
                            nc.vector.tensor_copy(ysb[:], psy[:])
                        else:
                            nc.scalar.activation(
                                ysb[:], psy[:], mybir.ActivationFunctionType.Copy
                            )
                        nc.sync.dma_start(y_r[:, tb, bass.ts(cc, 512)], ysb[:])

    nc.compile()
    return nc


def kernel(x, Wq, Wk, Wv, Wo):
    global LAST_RESULTS, _NC_CACHE
    import ml_dtypes

    bfloat16 = ml_dtypes.bfloat16

    x = np.asarray(x, dtype=np.float32)
    Wq = np.asarray(Wq, dtype=np.float32)
    Wk = np.asarray(Wk, dtype=np.float32)
    Wv = np.asarray(Wv, dtype=np.float32)
    Wo = np.asarray(Wo, dtype=np.float32)

    slopes = np.asarray(get_slopes(NH), dtype=np.float32)
    ii = np.arange(T, dtype=np.float64)
    pp = np.arange(P, dtype=np.float64)

    if _NC_CACHE is None:
        _NC_CACHE = build_kernel()
    nc = _NC_CACHE

    in_maps = []
    for core in range(8):
        b, g = core // 2, core % 2
        hsl = slice(g * 512, (g + 1) * 512)
        core_slopes = slopes[g * NHC : (g + 1) * NHC].astype(np.float64)

        qaug1 = (-core_slopes[:, None] * ii[None, :]).astype(bfloat16)
        qaugb = np.ascontiguousarray(np.broadcast_to(qaug1[:, None, :], (8, NHC, T)))
        kaugb = np.zeros((8, NHC, T), bfloat16)
        for h in range(NHC):
            kaugb[h, h, :] = bfloat16(1.0)
        biasj = np.zeros((P, NHC, NJB), np.float32)
        for h in range(NHC):
            for jb in range(NJB):
                biasj[:, h, jb] = (core_slopes[h] * (P * jb + pp)).astype(np.float32)
        in_maps.append(
            {
                "xb": np.ascontiguousarray(x[b].T).astype(bfloat16),
                "wq": (np.ascontiguousarray(Wq[:, hsl]) * np.float32(0.125)).astype(
                    bfloat16
                ),
                "wk": np.ascontiguousarray(Wk[:, hsl]).astype(bfloat16),
                "wv": np.ascontiguousarray(Wv[:, hsl]).astype(bfloat16),
                "wo": np.ascontiguousarray(Wo[hsl, :]).astype(bfloat16),
                "qaugb": qaugb,
                "kaugb": kaugb,
                "biasj": biasj,
            }
        )

    res = run_bass_kernel_spmd(nc, in_maps, list(range(8)))
    LAST_RESULTS = res
    out = np.empty((B, T, C), dtype=np.float32)
    for b in range(B):
        out[b] = res.results[2 * b]["y"].astype(np.float32) + res.results[2 * b + 1][
            "y"
        ].astype(np.float32)
    return out


# revision 5
# speedup vs baseline: 1.0311x; 1.0311x over previous
"""Causal attention with ALiBi for nn_CausalAttention (B=4, T=2048, C=1024,
16 heads) on 8 TRN2 NeuronCores.

Sharding: batch (4) x head-group (2 groups of 8 heads) -> 8 cores.
Host pre-casts x and weights to bf16 (and folds the 1/8 logit scale into
Wq), so the device does no casts and loads x exactly once.

Per core, three phases:
  1. Projections: one pass over x in two 1024-t chunks; per chunk compute
     q/k for all 4 head-pairs (1024-col matmuls amortize LDWEIGHTS) and v,
     writing qT/kT in [d, t] layout (aug rows 64-71 carry the one-hot /
     -slope*i ALiBi trick; the -slope*i bf16 error cancels exactly in
     softmax) and v in [t, h, d] layout.
  2. Attention, head-major with lag-1 PV: per (head, j-block) QK is
     computed only over the causal span i >= 128*jb, exp'd in <=1536-col
     groups (3 PSUM banks) with the exact +slope*j f32 bias. The 128x128
     diagonal block is masked on DVE with min(exp, dmask) where dmask is
     +Inf on/below the causal boundary and 0 above (min(Inf,0)=0 also
     kills overflow from masked logits). PV uses column-offset partial
     matmuls in two 1024-wide halves; an appended ones column yields the
     softmax denominator. Normalization is batched per head: one
     [1,2048]->[128,16] DMA transpose + reciprocal + one
     partition_broadcast + one multiply.
  3. Output projection y_partial = oT.T @ Wo_rows, emitted in bf16;
     host sums the two head-group partials per batch in f32.
"""

import math

import numpy as np

import concourse.bass as bass
import concourse.mybir as mybir
import concourse.tile as tile
from concourse import bacc
from concourse.bass_utils import run_bass_kernel_spmd

B, T, C = 4, 2048, 1024
NH, HD = 16, 64
NHC = 8  # heads per core
NJB = T // 128  # 16 j-blocks
P = 128

f32 = mybir.dt.float32
bf16 = mybir.dt.bfloat16

LAST_RESULTS = None
_NC_CACHE = None


def get_slopes(n):
    def pow2(n):
        start = 2 ** (-(2 ** (-(math.log2(n) - 3))))
        return [start * start**i for i in range(n)]

    if math.log2(n).is_integer():
        return pow2(n)
    c = 2 ** math.floor(math.log2(n))
    return pow2(c) + get_slopes(2 * c)[0::2][: n - c]


# packed pT row offsets: row jb holds the causal span i in [128*jb, T),
# i.e. (16-jb)*128 columns, stored back-to-back.
OFFS = []
_o = 0
for _jb in range(NJB):
    OFFS.append(_o)
    _o += (NJB - _jb) * P
NPCOL = _o  # 17408


def build_kernel():
    nc = bacc.Bacc("TRN2", target_bir_lowering=False, debug=False, num_devices=8)

    xT_d = nc.dram_tensor("xb", [C, T], bf16, kind="ExternalInput").ap()
    wq_d = nc.dram_tensor("wq", [C, 512], bf16, kind="ExternalInput").ap()
    wk_d = nc.dram_tensor("wk", [C, 512], bf16, kind="ExternalInput").ap()
    wv_d = nc.dram_tensor("wv", [C, 512], bf16, kind="ExternalInput").ap()
    wo_d = nc.dram_tensor("wo", [512, C], bf16, kind="ExternalInput").ap()
    qaug_d = nc.dram_tensor("qaugb", [8, NHC, T], bf16, kind="ExternalInput").ap()
    kaug_d = nc.dram_tensor("kaugb", [8, NHC, T], bf16, kind="ExternalInput").ap()
    biasj_d = nc.dram_tensor("biasj", [P, NHC, NJB], f32, kind="ExternalInput").ap()
    y_d = nc.dram_tensor("y", [T, C], bf16, kind="ExternalOutput").ap()

    xT_r = xT_d.rearrange("(cb p) t -> p cb t", p=P)  # [128, 8, 2048]
    wq_r = wq_d.rearrange("(cb p) m -> p cb m", p=P)  # [128, 8, 512]
    wk_r = wk_d.rearrange("(cb p) m -> p cb m", p=P)
    wv_r = wv_d.rearrange("(cb p) m -> p cb m", p=P)
    wo_r = wo_d.rearrange("(mb p) n -> p mb n", p=P)  # [128, 4, 1024]
    y_r = y_d.rearrange("(tb p) c -> p tb c", p=P)  # [128, 16, 1024]

    with tile.TileContext(nc) as tc:
        with tc.tile_pool(name="persist", bufs=1) as persist:
            # ---- persistent tiles ----
            qT2 = persist.tile([72, NHC, T], bf16)
            kT2 = persist.tile([72, NHC, T], bf16)
            vaug = persist.tile([P, NJB, NHC, 66], bf16)
            # oT2[dh, head-pair m, i-half, i] ; partitions 0-63 head 2m,
            # 64-127 head 2m+1
            oT2 = persist.tile([P, 4, 4, 512], bf16)
            biasj = persist.tile([P, NHC, NJB], f32)
            wo_t = persist.tile([P, 4, C], bf16)
            dmask = persist.tile([P, P], bf16)

            nc.gpsimd.memset(vaug[:, :, :, 64:66], 1.0)
            nc.gpsimd.memset(dmask[:], 3.0e38)
            # dmask[p, f] = 3e38 where f >= p (keep), else 0 (mask)
            nc.gpsimd.affine_select(
                dmask[:],
                dmask[:],
                pattern=[[1, P]],
                compare_op=mybir.AluOpType.is_ge,
                fill=0.0,
                base=0,
                channel_multiplier=-1,
            )
            nc.sync.dma_start(biasj[:], biasj_d[:])
            nc.sync.dma_start(kT2[64:72, :, :], kaug_d[:])
            nc.sync.dma_start(qT2[64:72, :, :], qaug_d[:])
            nc.sync.dma_start(wo_t[:], wo_r[:])

            # ---- phase 1: projections (single pass over x) ----
            with (
                tc.tile_pool(name="wp", bufs=1) as wp,
                tc.tile_pool(name="xp", bufs=16) as xp,
                tc.tile_pool(name="psQK", bufs=4, space="PSUM") as psQK,
                tc.tile_pool(name="psV", bufs=2, space="PSUM") as psV,
            ):
                wq_t = wp.tile([P, 8, 512], bf16)
                wk_t = wp.tile([P, 8, 512], bf16)
                wv_t = wp.tile([P, 8, 512], bf16)
                nc.sync.dma_start(wq_t[:], wq_r[:])
                nc.sync.dma_start(wk_t[:], wk_r[:])
                nc.sync.dma_start(wv_t[:], wv_r[:])

                for tck in range(2):
                    tsl = bass.ts(tck, 1024)
                    xts = []
                    for c in range(8):
                        xt = xp.tile([P, 1024], bf16, tag="xt")
                        nc.sync.dma_start(xt[:], xT_r[:, c, tsl])
                        xts.append(xt)
                    for m in range(4):
                        for hh in range(2):
                            ts2 = bass.ts(2 * tck + hh, 512)
                            hsl = bass.ts(hh, 512)
                            psq = psQK.tile([P, 512], f32, tag="pqk")
                            psk = psQK.tile([P, 512], f32, tag="pqk")
                            for c in range(8):
                                nc.tensor.matmul(
                                    psq[:],
                                    wq_t[:, c, bass.ts(m, P)],
                                    xts[c][:, hsl],
                                    start=(c == 0),
                                    stop=(c == 7),
                                )
                                nc.tensor.matmul(
                                    psk[:],
                                    wk_t[:, c, bass.ts(m, P)],
                                    xts[c][:, hsl],
                                    start=(c == 0),
                                    stop=(c == 7),
                                )
                            # even head: aligned ACT copy; odd head: DVE
                            # partition-shift copy (64-ch op may read any
                            # 64-part window, writes parts 0-63)
                            nc.scalar.activation(
                                qT2[0:64, 2 * m, ts2],
                                psq[0:64, :],
                                mybir.ActivationFunctionType.Copy,
                            )
                            nc.vector.tensor_copy(
                                qT2[0:64, 2 * m + 1, ts2], psq[64:128, :]
                            )
                            nc.scalar.activation(
                                kT2[0:64, 2 * m, ts2],
                                psk[0:64, :],
                                mybir.ActivationFunctionType.Copy,
                            )
                            nc.vector.tensor_copy(
                                kT2[0:64, 2 * m + 1, ts2], psk[64:128, :]
                            )
                    for tb in range(8):
                        psv = psV.tile([P, 512], f32, tag="pv")
                        for c in range(8):
                            nc.tensor.matmul(
                                psv[:],
                                xts[c][:, bass.ts(tb, P)],
                                wv_t[:, c, :],
                                start=(c == 0),
                                stop=(c == 7),
                            )
                        nc.vector.tensor_copy(
                            vaug[:, 8 * tck + tb, :, 0:64],
                            psv[:].rearrange("p (h d) -> p h d", h=NHC),
                        )

            # ---- phase 2: attention, head-major, lag-1 PV ----
            with (
                tc.tile_pool(name="psA", bufs=2, space="PSUM") as psA,
                tc.tile_pool(name="potp", bufs=2, space="PSUM") as potp,
                tc.tile_pool(name="pTp", bufs=2) as pTp,
                tc.tile_pool(name="pohp", bufs=2) as pohp,
                tc.tile_pool(name="bcp", bufs=1) as bcp,
                tc.tile_pool(name="srp", bufs=1) as srp,
                tc.tile_pool(name="rsp", bufs=2) as rsp,
            ):
                pT_of = {}

                def emit_qk(h):
                    pT = pTp.tile([P, NPCOL], bf16, tag="pT")
                    pT_of[h] = pT
                    for jb in range(NJB):
                        span = (NJB - jb) * P
                        off = 0
                        while off < span:
                            gw = min(1536, span - off)
                            ps = psA.tile([P, 1536], f32, tag="qk")
                            s = 0
                            while s < gw:
                                w = min(512, gw - s)
                                nc.tensor.matmul(
                                    ps[:, s : s + w],
                                    kT2[:, h, bass.ts(jb, P)],
                                    qT2[:, h, P * jb + off + s : P * jb + off + s + w],
                                    start=True,
                                    stop=True,
                                )
                                s += w
                            nc.scalar.activation(
                                pT[:, OFFS[jb] + off : OFFS[jb] + off + gw],
                                ps[:, 0:gw],
                                mybir.ActivationFunctionType.Exp,
                                bias=biasj[:, h, jb : jb + 1],
                                scale=1.0,
                            )
                            off += gw
                        # mask the 128x128 diagonal block on DVE:
                        # min(exp, dmask) zeroes f < p (and kills Inf)
                        nc.vector.tensor_tensor(
                            pT[:, OFFS[jb] : OFFS[jb] + P],
                            pT[:, OFFS[jb] : OFFS[jb] + P],
                            dmask[:],
                            mybir.AluOpType.min,
                        )

                def emit_pv(h):
                    hp = (h % 2) * 64
                    hm = h // 2
                    pT = pT_of.pop(h)
                    poh = pohp.tile([65, 4, 512], f32, tag="poh")
                    for c in range(4):
                        pot = potp.tile([65, 512], f32, tag="pot")
                        njb = 4 * c + 4
                        for jb in range(njb):
                            w = min(512, (njb - jb) * P)
                            roff = 512 * c - P * jb if jb <= 4 * c else 0
                            nc.tensor.matmul(
                                pot[:, 512 - w : 512],
                                vaug[:, jb, h, 0:65],
                                pT[:, OFFS[jb] + roff : OFFS[jb] + roff + w],
                                start=(jb == 0),
                                stop=(jb == njb - 1),
                            )
                        nc.vector.tensor_copy(poh[:, c, :], pot[:])
                    # batched normalization for the whole head
                    rs = rsp.tile([P, 16], f32, tag="rs")
                    nc.gpsimd.dma_start(rs[:], poh[64:65, :, :])
                    nc.vector.reciprocal(rs[:], rs[:])
                    sr = srp.tile([1, T], f32, tag="sr")
                    nc.gpsimd.dma_start(sr[:], rs[:])
                    bc = bcp.tile([64, 4, 512], f32, tag="bc")
                    nc.gpsimd.partition_broadcast(bc[:], sr[0:1, :])
                    nc.vector.tensor_tensor(
                        oT2[hp : hp + 64, hm, :, :],
                        poh[0:64, :, :],
                        bc[:],
                        mybir.AluOpType.mult,
                    )

                for h in range(NHC):
                    emit_qk(h)
                    if h > 0:
                        emit_pv(h - 1)
                emit_pv(NHC - 1)

            # ---- phase 3: output projection ----
            with (
                tc.tile_pool(name="psY", bufs=2, space="PSUM") as psY,
                tc.tile_pool(name="ypool", bufs=4) as ypool,
            ):
                for tb in range(NJB):
                    ysb = ypool.tile([P, 1024], bf16, tag="ysb")
                    for cc in range(2):
                        psy = psY.tile([P, 512], f32, tag="py")
                        for m in range(4):
                            nc.tensor.matmul(
                                psy[:],
                                oT2[:, m, tb // 4, bass.ts(tb % 4, P)],
                                wo_t[:, m, bass.ts(cc, 512)],
                                start=(m == 0),
                                stop=(m == 3),
                            )
                        if cc == 0:
                            nc.vector.tensor_copy(ysb[:, 0:512], psy[:])
                        else:
                            nc.scalar.activation(
                                ysb[:, 512:1024],
                                psy[:],
                                mybir.ActivationFunctionType.Copy,
                            )
                    nc.sync.dma_start(y_r[:, tb, :], ysb[:])

    nc.compile()
    return nc


def kernel(x, Wq, Wk, Wv, Wo):
    global LAST_RESULTS, _NC_CACHE
    import ml_dtypes

    bfloat16 = ml_dtypes.bfloat16

    x = np.asarray(x, dtype=np.float32)
    Wq = np.asarray(Wq, dtype=np.float32)
    Wk = np.asarray(Wk, dtype=np.float32)
    Wv = np.asarray(Wv, dtype=np.float32)
    Wo = np.asarray(Wo, dtype=np.float32)

    slopes = np.asarray(get_slopes(NH), dtype=np.float32)
    ii = np.arange(T, dtype=np.float64)
    pp = np.arange(P, dtype=np.float64)

    if _NC_CACHE is None:
        _NC_CACHE = build_kernel()
    nc = _NC_CACHE

    in_maps = []
    for core in range(8):
        b, g = core // 2, core % 2
        hsl = slice(g * 512, (g + 1) * 512)
        core_slopes = slopes[g * NHC : (g + 1) * NHC].astype(np.float64)

        qaug1 = (-core_slopes[:, None] * ii[None, :]).astype(bfloat16)
        qaugb = np.ascontiguousarray(np.broadcast_to(qaug1[:, None, :], (8, NHC, T)))
        kaugb = np.zeros((8, NHC, T), bfloat16)
        for h in range(NHC):
            kaugb[h, h, :] = bfloat16(1.0)
        biasj = np.zeros((P, NHC, NJB), np.float32)
        for h in range(NHC):
            for jb in range(NJB):
                biasj[:, h, jb] = (core_slopes[h] * (P * jb + pp)).astype(np.float32)
        in_maps.append(
            {
                "xb": np.ascontiguousarray(x[b].T).astype(bfloat16),
                "wq": (np.ascontiguousarray(Wq[:, hsl]) * np.float32(0.125)).astype(
                    bfloat16
                ),
                "wk": np.ascontiguousarray(Wk[:, hsl]).astype(bfloat16),
                "wv": np.ascontiguousarray(Wv[:, hsl]).astype(bfloat16),
                "wo": np.ascontiguousarray(Wo[hsl, :]).astype(bfloat16),
                "qaugb": qaugb,
                "kaugb": kaugb,
                "biasj": biasj,
            }
        )

    res = run_bass_kernel_spmd(nc, in_maps, list(range(8)))
    LAST_RESULTS = res
    out = np.empty((B, T, C), dtype=np.float32)
    for b in range(B):
        out[b] = res.results[2 * b]["y"].astype(np.float32) + res.results[2 * b + 1][
            "y"
        ].astype(np.float32)
    return out
